# revision 17
# baseline (speedup 1.0000x reference)
"""CrossAttention Trainium2 kernel.

Full-input contract: kernel(**inputs) takes the unsharded tensors
(x [32,1024,640], y [32,77,768], Wq,bq,Wk,bk,Wv,bv,Wo,bo) and returns
the full [32,1024,640] output.  Internally: data-parallel over batch
across 8 NeuronCores (4 batches per core), one shared SPMD Bass/Tile
kernel, no collectives.

Key design points (v2, packed-640 "zoned" layout):
  * x and y are transposed on the HOST (free) -> no PE transposes.
  * All tensors use the packed 640-wide (h,d) layout, no 96-padding:
    Q proj is 25 full [128x128xK] matmuls per 512-q block (vs 40
    padded), out proj 40 (vs 64).
  * Per-head isolation for S (scores) and O (attn@V) is done with
    zero-stuffed "zones": head h's 80 rows live at packed partition
    offsets 80h..80h+80, crossing 128-chunk boundaries for h=1,3,4,6.
    Each (head, chunk) incidence is a zone; the stationary operand
    (kt / v) is materialized per zone with zeros outside the head's
    rows, so every matmul AP stays at partition base 0.
  * bk is dropped exactly (softmax is invariant to per-q shifts);
    bv is folded into bo exactly (softmax weights sum to 1):
    bo_eff = bo + bv @ Wo.  K/V evacuations are plain copies.
  * Softmax normalizer: per 128-chunk of the packed dim, F and O are
    accumulated over the chunk's owner zones with zone-zero-stuffed
    stationaries (zoned ones / zoned v), so each chunk gets one
    ln + exp(-x) (ScalarE, shared ACT table set) and one full-width
    DVE multiply at partition base 0 (SBUF APs may only start at
    partition 0/32/64/96, so per-head partition slices are illegal).

Softmax needs no max subtraction: scores/sqrt(D) ~ N(0,1); max over
20M samples is ~6 sigma, far inside fp32 exp range.
"""

import os
import sys

import numpy as np

for _p in ("/opt/trn_rl_repo", os.path.expanduser("~/.axon_site/_ro/trn_rl_repo")):
    if os.path.isdir(_p) and _p not in sys.path:
        sys.path.insert(0, _p)
        break

# --- problem constants (hardcoded per contract) ---
B, SQ, SKV = 32, 1024, 77
E, C = 640, 768
H, D = 8, 80
N_CORES = 8
B_LOC = B // N_CORES   # 4
P = 128
QBLK = 512
EC = E // P            # 5 chunks over embed dim
CC = C // P            # 6 chunks over cross dim
NBLK = SQ // QBLK      # 2
SCALE = 1.0 / float(np.sqrt(D))

# zones: (head, chunk, offset-in-chunk, nrows, start-within-head-d)
ZONES = []
for _h in range(H):
    _start = D * _h
    _off = _start % P
    _c = _start // P
    if _off + D <= P:
        ZONES.append((_h, _c, _off, D, 0))
    else:
        _n1 = P - _off
        ZONES.append((_h, _c, _off, _n1, 0))
        ZONES.append((_h, _c + 1, 0, D - _n1, _n1))
NZ = len(ZONES)  # 12
ZONES_OF = [[i for i, z in enumerate(ZONES) if z[0] == h] for h in range(H)]
NZ_OF_CHUNK = [sum(1 for z in ZONES if z[1] == c) for c in range(EC)]

LAST_RESULTS = None  # BassKernelResults of the most recent run (for test.py)

_BUILT = None


def _build():
    """Build the SPMD Bass kernel once."""
    import concourse.bass as bass
    import concourse.bacc as bacc
    import concourse.mybir as mybir
    import concourse.tile as tile
    from contextlib import ExitStack

    f32 = mybir.dt.float32
    bf16 = mybir.dt.bfloat16
    AF = mybir.ActivationFunctionType
    ALU = mybir.AluOpType

    import bass_rust as _bass_rust
    from concourse.hw_specs import get_activation_tables

    class _Bacc(bacc.Bacc):
        # All our ACT functions (Exp, Ln, Copy, Identity) live in the
        # natural_log_exp_and_others set.  The stock greedy table-load pass
        # thrashes between exp_and_others and natural_log; blank every
        # other set so each ACTIVATE resolves to the one shared set.
        def insert_act_table_loads(self):
            has_activation = any(
                isinstance(i, mybir.InstActivation)
                for blk in self.main_func.blocks
                for i in blk.instructions
            )
            if not has_activation:
                return
            tables = [
                (name, funcs if name == "natural_log_exp_and_others" else set())
                for name, funcs in get_activation_tables(self.m.arch).items()
            ]
            _bass_rust.insert_act_table_loads(self, tables)

    nc = _Bacc("TRN2", target_bir_lowering=False, debug=False)

    x_d = nc.dram_tensor("x", [B_LOC, P, EC, SQ], bf16, kind="ExternalInput").ap()
    y_d = nc.dram_tensor("y", [P, CC, B_LOC, SKV], bf16, kind="ExternalInput").ap()
    wq_d = nc.dram_tensor("wq", [P, EC, E], bf16, kind="ExternalInput").ap()
    bq_d = nc.dram_tensor("bq", [P, EC], f32, kind="ExternalInput").ap()
    wk_d = nc.dram_tensor("wk", [P, CC, NZ * P], bf16, kind="ExternalInput").ap()
    wv_d = nc.dram_tensor("wv", [P, CC, E], bf16, kind="ExternalInput").ap()
    wo_d = nc.dram_tensor("wo", [P, EC, E], bf16, kind="ExternalInput").ap()
    bo_d = nc.dram_tensor("bo", [P, EC], f32, kind="ExternalInput").ap()
    ones_d = nc.dram_tensor("ones", [SKV, NZ, P], bf16, kind="ExternalInput").ap()
    out_d = nc.dram_tensor("out", [B_LOC, NBLK, EC, P, QBLK], f32, kind="ExternalOutput").ap()

    with tile.TileContext(nc) as tc, ExitStack() as ctx:
        const = ctx.enter_context(tc.tile_pool(name="const", bufs=1))
        wpool = ctx.enter_context(tc.tile_pool(name="wts", bufs=1))
        kvpool = ctx.enter_context(tc.tile_pool(name="kv", bufs=1))
        xtpool = ctx.enter_context(tc.tile_pool(name="xt", bufs=2))
        psQ = ctx.enter_context(tc.tile_pool(name="psQ", bufs=1, space="PSUM"))
        psS = ctx.enter_context(tc.tile_pool(name="psS", bufs=2, space="PSUM"))
        psF = ctx.enter_context(tc.tile_pool(name="psF", bufs=2, space="PSUM"))
        psO = ctx.enter_context(tc.tile_pool(name="psO", bufs=2, space="PSUM"))
        psout = ctx.enter_context(tc.tile_pool(name="psout", bufs=1, space="PSUM"))

        # y/K/V path on the scalar HWDGE queue, x/Q path on sync queue.
        yt = kvpool.tile([P, CC, B_LOC, SKV], bf16)
        nc.scalar.dma_start(yt[:], y_d)
        bq_s = const.tile([P, EC], f32)
        nc.sync.dma_start(bq_s[:], bq_d)

        kvw_ctx = ExitStack()
        kvwpool = kvw_ctx.enter_context(tc.tile_pool(name="kvw", bufs=1))
        wk_s = kvwpool.tile([P, CC, NZ * P], bf16)
        for piece in range(3):  # pipeline zone availability
            zsl = slice(piece * 4 * P, (piece + 1) * 4 * P)
            nc.scalar.dma_start(wk_s[:, :, zsl], wk_d[:, :, zsl])
        wv_s = kvwpool.tile([P, CC, E], bf16)
        nc.gpsimd.dma_start(wv_s[:], wv_d)
        ones_t = const.tile([SKV, NZ, P], bf16)
        nc.gpsimd.dma_start(ones_t[:], ones_d)
        bo_b = const.tile([P, EC], f32)
        nc.gpsimd.dma_start(bo_b[:], bo_d)

        wq_s = wpool.tile([P, EC, E], bf16)
        nc.sync.dma_start(wq_s[:], wq_d)
        wo_s = wpool.tile([P, EC, E], bf16)
        nc.gpsimd.dma_start(wo_s[:], wo_d)

        # ---- K projection (zoned) ----
        kt = kvpool.tile([P, NZ, B_LOC, SKV], bf16)
        for z in range(NZ):
            ps_k = psQ.tile([P, B_LOC, SKV], f32, tag="q")
            for c2 in range(CC):
                nc.tensor.matmul(
                    ps_k[:],
                    wk_s[:, c2, z * P : (z + 1) * P],
                    yt[:, c2],
                    start=(c2 == 0),
                    stop=(c2 == CC - 1),
                )
            nc.scalar.copy(kt[:, z], ps_k[:])

        # ---- V projection (packed) + on-chip zoning ----
        v_pack = kvwpool.tile([SKV, B_LOC, E], bf16)
        for b in range(B_LOC):
            for n, fsl in enumerate((slice(0, 384), slice(384, 640))):
                ps_v = psS.tile([SKV, 384], f32, tag="s")
                w = 384 if n == 0 else 256
                for c2 in range(CC):
                    nc.tensor.matmul(
                        ps_v[:, :w],
                        yt[:, c2, b, :],
                        wv_s[:, c2, fsl],
                        start=(c2 == 0),
                        stop=(c2 == CC - 1),
                    )
                nc.scalar.copy(v_pack[:, b, fsl], ps_v[:, :w])

        vz = kvpool.tile([SKV, B_LOC, NZ, P], bf16)
        nc.vector.memset(vz[:], 0.0)
        for b in range(B_LOC):
            for z, (h, c, off, n, s) in enumerate(ZONES):
                nc.vector.tensor_copy(
                    vz[:, b, z, off : off + n],
                    v_pack[:, b, D * h + s : D * h + s + n],
                )

        kvw_ctx.close()

        qpool = ctx.enter_context(tc.tile_pool(name="q", bufs=1))
        spool = ctx.enter_context(tc.tile_pool(name="s", bufs=2))
        rpool = ctx.enter_context(tc.tile_pool(name="r", bufs=2))
        apool = ctx.enter_context(tc.tile_pool(name="attn", bufs=2))
        opool = ctx.enter_context(tc.tile_pool(name="ost", bufs=3))

        # ---- main loop over local batches / q blocks ----
        for b in range(B_LOC):
            for blk in range(NBLK):
                qs = slice(blk * QBLK, (blk + 1) * QBLK)
                xt = xtpool.tile([P, EC, QBLK], bf16, tag="xt")
                nc.sync.dma_start(xt[:], x_d[b, :, :, qs])

                qt = qpool.tile([P, EC, QBLK], bf16)
                attn = apool.tile([P, EC, QBLK], bf16)

                def qproj(c):
                    ps_q = psQ.tile([P, QBLK], f32, tag="q")
                    for e in range(EC):
                        nc.tensor.matmul(
                            ps_q[:],
                            wq_s[:, e, c * P : (c + 1) * P],
                            xt[:, e, :],
                            start=(e == 0),
                            stop=(e == EC - 1),
                        )
                    nc.vector.tensor_tensor(
                        qt[:, c], ps_q[:],
                        bq_s[:, c : c + 1].to_broadcast([P, QBLK]), ALU.add,
                    )

                # per-chunk F/O accumulation state
                ps_fc = [None] * EC
                ps_oc = [None] * EC
                zdone = [0] * EC

                def head(h):
                    zs = ZONES_OF[h]
                    ps_s = psS.tile([SKV, QBLK], f32, tag="s")
                    for i, z in enumerate(zs):
                        nc.tensor.matmul(
                            ps_s[:],
                            kt[:, z, b, :],
                            qt[:, ZONES[z][1]],
                            start=(i == 0),
                            stop=(i == len(zs) - 1),
                        )
                    ew = spool.tile([SKV, QBLK], bf16, tag="ew")
                    nc.scalar.activation(ew[:], ps_s[:], AF.Exp)
                    for z in zs:
                        _, c, off, n, _ = ZONES[z]
                        first = zdone[c] == 0
                        last = zdone[c] + 1 == NZ_OF_CHUNK[c]
                        if first:
                            ps_fc[c] = psF.tile([P, QBLK], f32, tag="f", name="ps_fc")
                            ps_oc[c] = psO.tile([P, QBLK], f32, tag="o", name="ps_oc")
                        nc.tensor.matmul(
                            ps_fc[c][:], ones_t[:, z, :], ew[:],
                            start=first, stop=last,
                        )
                        nc.tensor.matmul(
                            ps_oc[c][:], vz[:, b, z, :], ew[:],
                            start=first, stop=last,
                        )
                        zdone[c] += 1
                        if last:
                            lnf = rpool.tile([P, QBLK], f32, tag="lnf")
                            nc.scalar.activation(lnf[:], ps_fc[c][:], AF.Ln)
                            rcf = rpool.tile([P, QBLK], f32, tag="rcf")
                            nc.scalar.activation(rcf[:], lnf[:], AF.Exp, scale=-1.0)
                            nc.vector.tensor_tensor(
                                attn[:, c, :], ps_oc[c][:], rcf[:], ALU.mult
                            )

                # interleave Q-proj chunks with heads as their chunks ready
                qproj(0)
                qproj(1)
                head(0)
                head(1)
                head(2)
                qproj(2)
                head(3)
                qproj(3)
                head(4)
                head(5)
                qproj(4)
                head(6)
                head(7)

                # transposed output projection: outT[e,q] per 128-e chunk
                for ec in range(EC):
                    ps1 = psout.tile([P, QBLK], f32, tag="m1")
                    ost = opool.tile([P, QBLK], f32, tag="ost")
                    for c in range(EC):
                        nc.tensor.matmul(
                            ps1[:], wo_s[:, c, ec * P : (ec + 1) * P], attn[:, c, :],
                            start=(c == 0), stop=(c == EC - 1),
                        )
                    nc.vector.tensor_tensor(
                        ost[:], ps1[:],
                        bo_b[:, ec : ec + 1].to_broadcast([P, QBLK]), ALU.add,
                    )
                    nc.gpsimd.dma_start(out_d[b, blk, ec], ost[:])

    nc.compile()
    return nc


def _get_built():
    global _BUILT
    if _BUILT is None:
        _BUILT = _build()
    return _BUILT


def kernel(x, y, Wq, bq, Wk, bk, Wv, bv, Wo, bo):
    global LAST_RESULTS
    from concourse.bass_utils import run_bass_kernel_spmd

    nc = _get_built()

    x = np.asarray(x, np.float32)
    y = np.asarray(y, np.float32)
    Wq = np.asarray(Wq, np.float32)
    bq_v = np.asarray(bq, np.float32)
    Wk = np.asarray(Wk, np.float32)
    Wv = np.asarray(Wv, np.float32)
    bv_v = np.asarray(bv, np.float32)
    Wo = np.asarray(Wo, np.float32)
    bo_v = np.asarray(bo, np.float32)

    wk_zoned = np.zeros((C, NZ, P), np.float32)
    for z, (h, c, off, n, s) in enumerate(ZONES):
        wk_zoned[:, z, off : off + n] = Wk[:, D * h + s : D * h + s + n]

    bo_eff = bo_v + bv_v @ Wo

    ones_zoned = np.zeros((SKV, NZ, P), np.float32)
    for z, (h, c, off, n, s) in enumerate(ZONES):
        ones_zoned[:, z, off : off + n] = 1.0

    shared = {
        "wq": (Wq * SCALE).reshape(EC, P, E).transpose(1, 0, 2),
        "bq": (bq_v * SCALE).reshape(EC, P).T,
        "wk": wk_zoned.reshape(CC, P, NZ * P).transpose(1, 0, 2),
        "wv": Wv.reshape(CC, P, E).transpose(1, 0, 2),
        "wo": Wo.reshape(EC, P, E).transpose(1, 0, 2),
        "bo": bo_eff.reshape(EC, P).T,
        "ones": ones_zoned,
    }
    import ml_dtypes
    bf = ml_dtypes.bfloat16
    f32_keys = {"bq", "bo"}
    shared = {
        k: np.ascontiguousarray(v, np.float32 if k in f32_keys else bf)
        for k, v in shared.items()
    }

    in_maps = []
    for core in range(N_CORES):
        bs = slice(core * B_LOC, (core + 1) * B_LOC)
        xt = np.ascontiguousarray(
            x[bs].reshape(B_LOC, SQ, EC, P).transpose(0, 3, 2, 1).astype(bf)
        )
        yt = np.ascontiguousarray(
            y[bs].reshape(B_LOC, SKV, CC, P).transpose(3, 2, 0, 1).astype(bf)
        )
        m = {"x": xt, "y": yt}
        m.update(shared)
        in_maps.append(m)

    res = run_bass_kernel_spmd(nc, in_maps, core_ids=list(range(N_CORES)))
    LAST_RESULTS = res

    out = np.empty((B, SQ, E), np.float32)
    for core in range(N_CORES):
        # [B_LOC, NBLK, EC, P, QBLK] -> [B_LOC, NBLK*QBLK(q), EC*P(e)]
        o = res.results[core]["out"].transpose(0, 1, 4, 2, 3).reshape(B_LOC, SQ, E)
        out[core * B_LOC : (core + 1) * B_LOC] = o
    return out


# revision 18
# speedup vs baseline: 1.0150x; 1.0150x over previous
"""CrossAttention Trainium2 kernel.

Full-input contract: kernel(**inputs) takes the unsharded tensors
(x [32,1024,640], y [32,77,768], Wq,bq,Wk,bk,Wv,bv,Wo,bo) and returns
the full [32,1024,640] output.  Internally: data-parallel over batch
across 8 NeuronCores (4 batches per core), one shared SPMD Bass/Tile
kernel, no collectives.

Key design points (v2, packed-640 "zoned" layout):
  * x and y are transposed on the HOST (free) -> no PE transposes.
  * All tensors use the packed 640-wide (h,d) layout, no 96-padding:
    Q proj is 25 full [128x128xK] matmuls per 512-q block (vs 40
    padded), out proj 40 (vs 64).
  * Per-head isolation for S (scores) and O (attn@V) is done with
    zero-stuffed "zones": head h's 80 rows live at packed partition
    offsets 80h..80h+80, crossing 128-chunk boundaries for h=1,3,4,6.
    Each (head, chunk) incidence is a zone; the stationary operand
    (kt / v) is materialized per zone with zeros outside the head's
    rows, so every matmul AP stays at partition base 0.
  * bk is dropped exactly (softmax is invariant to per-q shifts);
    bv is folded into bo exactly (softmax weights sum to 1):
    bo_eff = bo + bv @ Wo.  K/V evacuations are plain copies.
  * Softmax normalizer: per 128-chunk of the packed dim, F and O are
    accumulated over the chunk's owner zones with zone-zero-stuffed
    stationaries (zoned ones / zoned v), so each chunk gets one
    ln + exp(-x) (ScalarE, shared ACT table set) and one full-width
    DVE multiply at partition base 0 (SBUF APs may only start at
    partition 0/32/64/96, so per-head partition slices are illegal).

Softmax needs no max subtraction: scores/sqrt(D) ~ N(0,1); max over
20M samples is ~6 sigma, far inside fp32 exp range.
"""

import os
import sys

import numpy as np

for _p in ("/opt/trn_rl_repo", os.path.expanduser("~/.axon_site/_ro/trn_rl_repo")):
    if os.path.isdir(_p) and _p not in sys.path:
        sys.path.insert(0, _p)
        break

# --- problem constants (hardcoded per contract) ---
B, SQ, SKV = 32, 1024, 77
E, C = 640, 768
H, D = 8, 80
N_CORES = 8
B_LOC = B // N_CORES   # 4
P = 128
QBLK = 512
EC = E // P            # 5 chunks over embed dim
CC = C // P            # 6 chunks over cross dim
NBLK = SQ // QBLK      # 2
SCALE = 1.0 / float(np.sqrt(D))

# zones: (head, chunk, offset-in-chunk, nrows, start-within-head-d)
ZONES = []
for _h in range(H):
    _start = D * _h
    _off = _start % P
    _c = _start // P
    if _off + D <= P:
        ZONES.append((_h, _c, _off, D, 0))
    else:
        _n1 = P - _off
        ZONES.append((_h, _c, _off, _n1, 0))
        ZONES.append((_h, _c + 1, 0, D - _n1, _n1))
NZ = len(ZONES)  # 12
ZONES_OF = [[i for i, z in enumerate(ZONES) if z[0] == h] for h in range(H)]
NZ_OF_CHUNK = [sum(1 for z in ZONES if z[1] == c) for c in range(EC)]

LAST_RESULTS = None  # BassKernelResults of the most recent run (for test.py)

_BUILT = None


def _build():
    """Build the SPMD Bass kernel once."""
    import concourse.bass as bass
    import concourse.bacc as bacc
    import concourse.mybir as mybir
    import concourse.tile as tile
    from contextlib import ExitStack

    f32 = mybir.dt.float32
    bf16 = mybir.dt.bfloat16
    AF = mybir.ActivationFunctionType
    ALU = mybir.AluOpType

    import bass_rust as _bass_rust
    from concourse.hw_specs import get_activation_tables

    class _Bacc(bacc.Bacc):
        # All our ACT functions (Exp, Ln, Copy, Identity) live in the
        # natural_log_exp_and_others set.  The stock greedy table-load pass
        # thrashes between exp_and_others and natural_log; blank every
        # other set so each ACTIVATE resolves to the one shared set.
        def insert_act_table_loads(self):
            has_activation = any(
                isinstance(i, mybir.InstActivation)
                for blk in self.main_func.blocks
                for i in blk.instructions
            )
            if not has_activation:
                return
            tables = [
                (name, funcs if name == "natural_log_exp_and_others" else set())
                for name, funcs in get_activation_tables(self.m.arch).items()
            ]
            _bass_rust.insert_act_table_loads(self, tables)

    nc = _Bacc("TRN2", target_bir_lowering=False, debug=False)

    x_d = nc.dram_tensor("x", [B_LOC, P, EC, SQ], bf16, kind="ExternalInput").ap()
    y_d = nc.dram_tensor("y", [P, CC, B_LOC, SKV], bf16, kind="ExternalInput").ap()
    wq_d = nc.dram_tensor("wq", [P, EC, E], bf16, kind="ExternalInput").ap()
    bq_d = nc.dram_tensor("bq", [P, EC], f32, kind="ExternalInput").ap()
    wk_d = nc.dram_tensor("wk", [P, CC, NZ * P], bf16, kind="ExternalInput").ap()
    wv_d = nc.dram_tensor("wv", [P, CC, E], bf16, kind="ExternalInput").ap()
    wo_d = nc.dram_tensor("wo", [P, EC, E], bf16, kind="ExternalInput").ap()
    bo_d = nc.dram_tensor("bo", [P, EC], f32, kind="ExternalInput").ap()
    ones_d = nc.dram_tensor("ones", [SKV, NZ, P], bf16, kind="ExternalInput").ap()
    out_d = nc.dram_tensor("out", [B_LOC, NBLK, EC, P, QBLK], f32, kind="ExternalOutput").ap()

    with tile.TileContext(nc) as tc, ExitStack() as ctx:
        const = ctx.enter_context(tc.tile_pool(name="const", bufs=1))
        wpool = ctx.enter_context(tc.tile_pool(name="wts", bufs=1))
        kvpool = ctx.enter_context(tc.tile_pool(name="kv", bufs=1))
        xtpool = ctx.enter_context(tc.tile_pool(name="xt", bufs=2))
        psQ = ctx.enter_context(tc.tile_pool(name="psQ", bufs=1, space="PSUM"))
        psS = ctx.enter_context(tc.tile_pool(name="psS", bufs=2, space="PSUM"))
        psF = ctx.enter_context(tc.tile_pool(name="psF", bufs=2, space="PSUM"))
        psO = ctx.enter_context(tc.tile_pool(name="psO", bufs=2, space="PSUM"))
        psout = ctx.enter_context(tc.tile_pool(name="psout", bufs=1, space="PSUM"))

        # y/K/V path on the scalar HWDGE queue, x/Q path on sync queue.
        yt = kvpool.tile([P, CC, B_LOC, SKV], bf16)
        nc.scalar.dma_start(yt[:], y_d)
        bq_s = const.tile([P, EC], f32)
        nc.sync.dma_start(bq_s[:], bq_d)

        kvw_ctx = ExitStack()
        kvwpool = kvw_ctx.enter_context(tc.tile_pool(name="kvw", bufs=1))
        wk_s = kvwpool.tile([P, CC, NZ * P], bf16)
        for piece in range(6):  # pipeline zone availability
            zsl = slice(piece * 2 * P, (piece + 1) * 2 * P)
            nc.scalar.dma_start(wk_s[:, :, zsl], wk_d[:, :, zsl])
        wv_s = kvwpool.tile([P, CC, E], bf16)
        nc.scalar.dma_start(wv_s[:], wv_d)
        ones_t = const.tile([SKV, NZ, P], bf16)
        nc.gpsimd.dma_start(ones_t[:], ones_d)
        bo_b = const.tile([P, EC], f32)
        nc.gpsimd.dma_start(bo_b[:], bo_d)

        wq_s = wpool.tile([P, EC, E], bf16)
        nc.sync.dma_start(wq_s[:], wq_d)
        wo_s = wpool.tile([P, EC, E], bf16)
        nc.gpsimd.dma_start(wo_s[:], wo_d)

        # ---- K projection (zoned) ----
        kt = kvpool.tile([P, NZ, B_LOC, SKV], bf16)
        for z in range(NZ):
            ps_k = psQ.tile([P, B_LOC, SKV], f32, tag="q")
            for c2 in range(CC):
                nc.tensor.matmul(
                    ps_k[:],
                    wk_s[:, c2, z * P : (z + 1) * P],
                    yt[:, c2],
                    start=(c2 == 0),
                    stop=(c2 == CC - 1),
                )
            nc.scalar.copy(kt[:, z], ps_k[:])

        # ---- V projection (packed) + on-chip zoning ----
        v_pack = kvwpool.tile([SKV, B_LOC, E], bf16)
        for b in range(B_LOC):
            for n, fsl in enumerate((slice(0, 384), slice(384, 640))):
                ps_v = psS.tile([SKV, 384], f32, tag="s")
                w = 384 if n == 0 else 256
                for c2 in range(CC):
                    nc.tensor.matmul(
                        ps_v[:, :w],
                        yt[:, c2, b, :],
                        wv_s[:, c2, fsl],
                        start=(c2 == 0),
                        stop=(c2 == CC - 1),
                    )
                nc.scalar.copy(v_pack[:, b, fsl], ps_v[:, :w])

        vz = kvpool.tile([SKV, B_LOC, NZ, P], bf16)
        nc.vector.memset(vz[:], 0.0)
        for b in range(B_LOC):
            for z, (h, c, off, n, s) in enumerate(ZONES):
                nc.vector.tensor_copy(
                    vz[:, b, z, off : off + n],
                    v_pack[:, b, D * h + s : D * h + s + n],
                )

        kvw_ctx.close()

        qpool = ctx.enter_context(tc.tile_pool(name="q", bufs=2))
        spool = ctx.enter_context(tc.tile_pool(name="s", bufs=3))
        rpool = ctx.enter_context(tc.tile_pool(name="r", bufs=3))
        apool = ctx.enter_context(tc.tile_pool(name="attn", bufs=2))
        opool = ctx.enter_context(tc.tile_pool(name="ost", bufs=3))

        # ---- main loop over local batches / q blocks ----
        for b in range(B_LOC):
            for blk in range(NBLK):
                qs = slice(blk * QBLK, (blk + 1) * QBLK)
                xt = xtpool.tile([P, EC, QBLK], bf16, tag="xt")
                nc.sync.dma_start(xt[:], x_d[b, :, :, qs])

                qt = qpool.tile([P, EC, QBLK], bf16)
                attn = apool.tile([P, EC, QBLK], bf16)

                def qproj(c):
                    ps_q = psQ.tile([P, QBLK], f32, tag="q")
                    for e in range(EC):
                        nc.tensor.matmul(
                            ps_q[:],
                            wq_s[:, e, c * P : (c + 1) * P],
                            xt[:, e, :],
                            start=(e == 0),
                            stop=(e == EC - 1),
                        )
                    nc.vector.tensor_tensor(
                        qt[:, c], ps_q[:],
                        bq_s[:, c : c + 1].to_broadcast([P, QBLK]), ALU.add,
                    )

                # per-chunk F/O accumulation state
                ps_fc = [None] * EC
                ps_oc = [None] * EC
                zdone = [0] * EC

                def head(h):
                    zs = ZONES_OF[h]
                    ps_s = psS.tile([SKV, QBLK], f32, tag="s")
                    for i, z in enumerate(zs):
                        nc.tensor.matmul(
                            ps_s[:],
                            kt[:, z, b, :],
                            qt[:, ZONES[z][1]],
                            start=(i == 0),
                            stop=(i == len(zs) - 1),
                        )
                    ew = spool.tile([SKV, QBLK], bf16, tag="ew")
                    nc.scalar.activation(ew[:], ps_s[:], AF.Exp)
                    for z in zs:
                        _, c, off, n, _ = ZONES[z]
                        first = zdone[c] == 0
                        last = zdone[c] + 1 == NZ_OF_CHUNK[c]
                        if first:
                            ps_fc[c] = psF.tile([P, QBLK], f32, tag="f", name="ps_fc")
                            ps_oc[c] = psO.tile([P, QBLK], f32, tag="o", name="ps_oc")
                        nc.tensor.matmul(
                            ps_fc[c][:], ones_t[:, z, :], ew[:],
                            start=first, stop=last,
                        )
                        nc.tensor.matmul(
                            ps_oc[c][:], vz[:, b, z, :], ew[:],
                            start=first, stop=last,
                        )
                        zdone[c] += 1
                        if last:
                            lnf = rpool.tile([P, QBLK], f32, tag="lnf")
                            nc.scalar.activation(lnf[:], ps_fc[c][:], AF.Ln)
                            rcf = rpool.tile([P, QBLK], f32, tag="rcf")
                            nc.scalar.activation(rcf[:], lnf[:], AF.Exp, scale=-1.0)
                            nc.vector.tensor_tensor(
                                attn[:, c, :], ps_oc[c][:], rcf[:], ALU.mult
                            )

                # interleave Q-proj chunks with heads as their chunks ready
                qproj(0)
                qproj(1)
                head(0)
                head(1)
                head(2)
                qproj(2)
                head(3)
                qproj(3)
                head(4)
                head(5)
                qproj(4)
                head(6)
                head(7)

                # transposed output projection: outT[e,q] per 128-e chunk
                last = b == B_LOC - 1 and blk == NBLK - 1
                for ec in range(EC):
                    pool = psQ if (last and ec % 2 == 1) else psout
                    ps1 = pool.tile([P, QBLK], f32, tag="q" if (last and ec % 2 == 1) else "m1", name="ps_ot")
                    ost = opool.tile([P, QBLK], f32, tag="ost")
                    for c in range(EC):
                        nc.tensor.matmul(
                            ps1[:], wo_s[:, c, ec * P : (ec + 1) * P], attn[:, c, :],
                            start=(c == 0), stop=(c == EC - 1),
                        )
                    nc.vector.tensor_tensor(
                        ost[:], ps1[:],
                        bo_b[:, ec : ec + 1].to_broadcast([P, QBLK]), ALU.add,
                    )
                    nc.gpsimd.dma_start(out_d[b, blk, ec], ost[:])

    nc.compile()
    return nc


def _get_built():
    global _BUILT
    if _BUILT is None:
        _BUILT = _build()
    return _BUILT


def kernel(x, y, Wq, bq, Wk, bk, Wv, bv, Wo, bo):
    global LAST_RESULTS
    from concourse.bass_utils import run_bass_kernel_spmd

    nc = _get_built()

    x = np.asarray(x, np.float32)
    y = np.asarray(y, np.float32)
    Wq = np.asarray(Wq, np.float32)
    bq_v = np.asarray(bq, np.float32)
    Wk = np.asarray(Wk, np.float32)
    Wv = np.asarray(Wv, np.float32)
    bv_v = np.asarray(bv, np.float32)
    Wo = np.asarray(Wo, np.float32)
    bo_v = np.asarray(bo, np.float32)

    wk_zoned = np.zeros((C, NZ, P), np.float32)
    for z, (h, c, off, n, s) in enumerate(ZONES):
        wk_zoned[:, z, off : off + n] = Wk[:, D * h + s : D * h + s + n]

    bo_eff = bo_v + bv_v @ Wo

    ones_zoned = np.zeros((SKV, NZ, P), np.float32)
    for z, (h, c, off, n, s) in enumerate(ZONES):
        ones_zoned[:, z, off : off + n] = 1.0

    shared = {
        "wq": (Wq * SCALE).reshape(EC, P, E).transpose(1, 0, 2),
        "bq": (bq_v * SCALE).reshape(EC, P).T,
        "wk": wk_zoned.reshape(CC, P, NZ * P).transpose(1, 0, 2),
        "wv": Wv.reshape(CC, P, E).transpose(1, 0, 2),
        "wo": Wo.reshape(EC, P, E).transpose(1, 0, 2),
        "bo": bo_eff.reshape(EC, P).T,
        "ones": ones_zoned,
    }
    import ml_dtypes
    bf = ml_dtypes.bfloat16
    f32_keys = {"bq", "bo"}
    shared = {
        k: np.ascontiguousarray(v, np.float32 if k in f32_keys else bf)
        for k, v in shared.items()
    }

    in_maps = []
    for core in range(N_CORES):
        bs = slice(core * B_LOC, (core + 1) * B_LOC)
        xt = np.ascontiguousarray(
            x[bs].reshape(B_LOC, SQ, EC, P).transpose(0, 3, 2, 1).astype(bf)
        )
        yt = np.ascontiguousarray(
            y[bs].reshape(B_LOC, SKV, CC, P).transpose(3, 2, 0, 1).astype(bf)
        )
        m = {"x": xt, "y": yt}
        m.update(shared)
        in_maps.append(m)

    res = run_bass_kernel_spmd(nc, in_maps, core_ids=list(range(N_CORES)))
    LAST_RESULTS = res

    out = np.empty((B, SQ, E), np.float32)
    for core in range(N_CORES):
        # [B_LOC, NBLK, EC, P, QBLK] -> [B_LOC, NBLK*QBLK(q), EC*P(e)]
        o = res.results[core]["out"].transpose(0, 1, 4, 2, 3).reshape(B_LOC, SQ, E)
        out[core * B_LOC : (core + 1) * B_LOC] = o
    return out
